# revision 31
# baseline (speedup 1.0000x reference)
"""Trainium2 Bass kernel for nn_AttentionModel (graph attention encoder + decoder).

Contract: kernel(**inputs) takes FULL unsharded numpy inputs (as produced by
reference.setup_inputs()) and returns the FULL [256, 100] float32 output.
Internally shards the batch (256) across 8 NeuronCores (32 each, pure data
parallel; weights replicated) and runs a fused Bass/Tile kernel per core.

Self-contained: hardcodes all shapes; no sibling imports.

v1 design notes (vs v0 baseline):
- Embedding MLPs done block-diagonally: one [20,304] input tile holds the
  per-type features on disjoint row blocks, so layer-1 is ONE matmul over all
  301 nodes; leaky-relu is done with native ACT Lrelu ops writing straight
  into the block-diagonal [99,304] hidden tile (rows 96:99 hold constant
  type-indicator rows so every per-type bias rides along as extra
  contraction rows).
- Layer-0 q/k/v are folded into the embedding layer-2 weights on the host
  (W2aug @ wq etc.), so h0T never exists; wv@wo is folded into a single
  v' = h @ (wv wo) everywhere (exact by associativity), which removes the
  output projection and lets attention output be computed naturally.
- Scores are computed TRANSPOSED (sT[k,q]) so exp(sT) IS the aT operand the
  AV matmul needs: no attention-matrix transposes. Row sums come from a
  ones-column matmul; 1/rowsum is applied per-q-row fused into the residual
  add (scalar_tensor_tensor).
- LN: bn_stats/bn_aggr stats (DVE), rstd = Exp(-0.5*Ln(var+eps)) (tiny ACT,
  keeps ALL activations in the single natural_log_exp table -> no act-table
  reloads), normalize on the otherwise-idle Pool engine.
- Final tanh is computed from exp (tanh z = 1 - 2/(e^{2z}+1)) to stay in the
  one act table.
- Copies psum->sbuf are balanced across ACT and DVE.
"""

import sys

for _p in ("/opt/trn_rl_repo", "/opt/pypackages"):
    if _p not in sys.path:
        sys.path.append(_p)

import numpy as np
from contextlib import ExitStack

# --- static architecture constants ---
B, IH, IL, LH, E, FFH, NL = 256, 200, 6, 100, 256, 512, 2
G = IH + LH + 1  # 301
CLIP = 10.0
SCALE = 1.0 / 16.0  # 1/sqrt(E)
NCORES = 8
BPC = B // NCORES  # 32 batch elements per core

GC = [(0, 128), (128, 256), (256, 301)]  # g chunks over 301 nodes
GCT = [(0, 128), (128, 256), (256, 302)]  # even-row chunks for f32r transposes
VN = 304  # padded moving width over the node axis
HID = 99  # 3*32 block-diag hidden rows + 3 bias-indicator rows


# ----------------------------------------------------------------------------
# host-side weight packing
# ----------------------------------------------------------------------------
def _tf32(x):
    """Round fp32 array to tfloat32 (10 mantissa bits), round-to-nearest-even."""
    u = np.ascontiguousarray(x, np.float32).view(np.uint32)
    u = (u + 0x0FFF + ((u >> 13) & 1)) & np.uint32(0xFFFFE000)
    return u.view(np.float32)


def _pack_rows(m, nchunk):
    """[nchunk*128, N] -> [128, nchunk, N] with [:, k, :] = m[128k:128(k+1), :]"""
    return np.ascontiguousarray(
        np.stack([m[i * 128:(i + 1) * 128] for i in range(nchunk)], axis=1)
    ).astype(np.float32)


def _prep_weights(inp):
    f32 = np.float32
    w = {}
    # block-diag layer 1: [20, 32] = [wi1; wl1; wn1], bias rows [3, 32]
    w["w1aug"] = np.concatenate(
        [inp["wi1"], inp["wl1"], inp["wn1"]], axis=0).astype(f32)  # [20, 32]
    w["b1s"] = np.stack(
        [inp["bi1"], inp["bl1"], inp["bn1"]], axis=0).astype(f32)  # [3, 32]
    # type indicator rows [3, 304] (1 on that type's node columns)
    bi = np.zeros((3, VN), f32)
    bi[0, :IH] = 1.0
    bi[1, IH:IH + LH] = 1.0
    bi[2, IH + LH:G] = 1.0
    w["bias_ind"] = bi
    # block-diag layer 2 (+ bias rows 96:99): [99, 256]
    w2aug = np.zeros((HID, E), f32)
    w2aug[0:32] = inp["wi2"]
    w2aug[32:64] = inp["wl2"]
    w2aug[64:96] = inp["wn2"]
    w2aug[96] = inp["bi2"]
    w2aug[97] = inp["bl2"]
    w2aug[98] = inp["bn2"]
    w["w2aug"] = _tf32(w2aug)
    # layer-0 q/k/v folded through the embedding layer 2
    wq0 = inp["enc_wq"][0] * SCALE
    wk0 = inp["enc_wk"][0]
    wv0 = inp["enc_wv"][0] @ inp["enc_wo"][0]
    w["wq0"] = _tf32(w2aug @ wq0)                               # [99, 256]
    w["wk0"] = _tf32(w2aug @ wk0)
    w["wv0"] = _tf32(w2aug @ wv0)
    # layer-1 projections (lhsT/rhs chunk packs)
    w["wq1"] = _tf32(_pack_rows(inp["enc_wq"][1] * SCALE, 2))   # [128,2,256]
    w["wk1"] = _tf32(_pack_rows(inp["enc_wk"][1], 2))
    w["wv1"] = _tf32(_pack_rows(inp["enc_wv"][1] @ inp["enc_wo"][1], 2))
    for l in range(NL):
        w[f"wf1{l}"] = _tf32(_pack_rows(inp["enc_wf1"][l], 2))  # [128,2,512]
        w[f"bf1{l}"] = np.ascontiguousarray(
            inp["enc_bf1"][l].reshape(4, 128).T).astype(f32)    # [128, 4]
        w[f"wf2{l}"] = _tf32(_pack_rows(inp["enc_wf2"][l], 4))  # [128,4,256]
        w[f"bf2{l}"] = _tf32(inp["enc_bf2"][l].reshape(1, E))
    # decoder fused matrix: compat = trans_leaf . (M @ ge), M = Wpn_E @ Wfc.T
    MT = (inp["w_fc"] @ inp["w_pn"][:, :E].T) * SCALE
    w["mt"] = _tf32(_pack_rows(MT, 2))                          # [128,2,256]
    w["zeros99"] = np.zeros((HID, VN), f32)
    return w


# ----------------------------------------------------------------------------
# numpy mirror of the device computation (for algebra validation)
# ----------------------------------------------------------------------------
def _numpy_mirror(observation, w):
    obs = observation.astype(np.float32)
    nb = obs.shape[0]
    out = np.zeros((nb, LH), np.float32)

    def lrelu(x):
        return np.maximum(x, 0.01 * x)

    def ln(x):
        m = x.mean(-1, keepdims=True)
        v = x.var(-1, keepdims=True)
        return (x - m) / np.sqrt(v + 1e-5)

    bi = w["bias_ind"][:, :G]  # [3, 301]
    for b in range(nb):
        # block-diagonal feature tile [20, 301]
        xa = np.zeros((20, G), np.float32)
        xa[0:6, :IH] = obs[b, :IH, :6].T
        xa[6:14, IH:IH + LH] = obs[b, IH:IH + LH, :8].T
        xa[14:20, IH + LH:] = obs[b, IH + LH:, :6].T
        z1 = w["w1aug"].T @ xa + w["b1s"].T @ bi               # [32, 301]
        lr = np.zeros((HID, G), np.float32)
        lr[0:32, :IH] = lrelu(z1[:, :IH])
        lr[32:64, IH:IH + LH] = lrelu(z1[:, IH:IH + LH])
        lr[64:96, IH + LH:] = lrelu(z1[:, IH + LH:])
        lr[96:99] = bi
        h = (lr.T @ w["w2aug"])                                 # [301, 256]

        for l in range(NL):
            if l == 0:
                qT = w["wq0"].T @ lr                            # [256, 301]
                kT = w["wk0"].T @ lr
                v = (lr.T @ w["wv0"])                           # [301, 256]
            else:
                wq = np.concatenate([w["wq1"][:, 0], w["wq1"][:, 1]], 0)
                wk = np.concatenate([w["wk1"][:, 0], w["wk1"][:, 1]], 0)
                wv = np.concatenate([w["wv1"][:, 0], w["wv1"][:, 1]], 0)
                qT = (h @ wq).T
                kT = (h @ wk).T
                v = h @ wv
            sT = kT.T @ qT                                      # [301k, 301q]
            aT = np.exp(sT)
            rinv = 1.0 / aT.sum(0)                              # per q
            o = (aT.T @ v) * rinv[:, None]                      # [301q, 256]
            x1 = h + o
            h = ln(x1)
            wf1 = np.concatenate([w[f"wf1{l}"][:, 0], w[f"wf1{l}"][:, 1]], 0)
            wf2 = np.concatenate([w[f"wf2{l}"][:, k] for k in range(4)], 0)
            bf1 = w[f"bf1{l}"].T.reshape(-1)
            f = np.maximum(h @ wf1 + bf1, 0.0) @ wf2 + w[f"bf2{l}"]
            h = ln(h + f)

        mask = obs[b, :, 8]
        trans = h * mask[:, None]
        ge = trans.sum(0)
        MT = np.concatenate([w["mt"][:, 0], w["mt"][:, 1]], 0)
        c = MT.T @ ge
        compat = trans[IH:IH + LH] @ c
        vlen = mask.sum()
        z = compat / vlen
        th = 1.0 - 2.0 / (np.exp(2.0 * z) + 1.0)                # tanh via exp
        ee = np.exp(th * CLIP)
        p = ee / ee.sum()
        lv = obs[b, IH:IH + LH, 8]
        masked = p * lv + 1e-20
        out[b] = masked / masked.sum()
    return out


# ----------------------------------------------------------------------------
# the Bass/Tile kernel
# ----------------------------------------------------------------------------
def _build(bpc, dbg=False, nbp=None, psb=(3, 2, 2, 1), has_bias=True, ceng=(1, 0), bufs=(3, 2, 5, 4)):
    import concourse.bass as bass
    import concourse.mybir as mybir
    import concourse.tile as tile
    from concourse import bacc
    from concourse.masks import make_identity

    f32 = mybir.dt.float32
    f32r = mybir.dt.float32r
    i32 = mybir.dt.int32
    AF = mybir.ActivationFunctionType
    ALU = mybir.AluOpType
    AX = mybir.AxisListType

    def r(ap):
        return ap.bitcast(f32r)

    nc = bacc.Bacc(None, target_bir_lowering=False)

    obs = nc.declare_dram_parameter("obs", [bpc, G, 9], f32, isOutput=False)
    dp = {}
    F32_WEIGHTS = {"w1aug", "b1s", "bias_ind", "bf10", "bf11", "mt"}
    for nm, shp in (
        ("w1aug", [20, 32]), ("b1s", [3, 32]), ("bias_ind", [3, VN]),
        ("w2aug", [HID, E]), ("wq0", [HID, E]), ("wk0", [HID, E]),
        ("wv0", [HID, E]),
        ("wq1", [128, 2, E]), ("wk1", [128, 2, E]), ("wv1", [128, 2, E]),
        ("wf10", [128, 2, FFH]), ("bf10", [128, 4]), ("wf20", [128, 4, E]),
        ("bf20", [1, E]),
        ("wf11", [128, 2, FFH]), ("bf11", [128, 4]), ("wf21", [128, 4, E]),
        ("bf21", [1, E]),
        ("mt", [128, 2, E]),  # fp32 (moving-1 matmuls)
        ("zeros99", [HID, VN]),
    ):
        wdt = f32 if nm in F32_WEIGHTS else f32r
        dp[nm] = nc.declare_dram_parameter(nm, shp, wdt, isOutput=False)
    out_d = nc.declare_dram_parameter("out", [bpc, LH], f32, isOutput=True)

    with tile.TileContext(nc) as tc, ExitStack() as ctx:
        const = ctx.enter_context(tc.tile_pool(name="const", bufs=1))
        ST_BUFS = bufs[0]
        st = ctx.enter_context(tc.tile_pool(name="st", bufs=ST_BUFS))
        stA = ctx.enter_context(tc.tile_pool(name="stA", bufs=bufs[1]))
        stH4 = ctx.enter_context(tc.tile_pool(name="stH4", bufs=bufs[2]))
        stH3 = ctx.enter_context(tc.tile_pool(name="stH3", bufs=bufs[3]))
        # Per-stage PSUM pools (8 banks total). A single shared-tag pool
        # makes every element's ~30 psum tiles queue FIFO behind each other,
        # serializing elements; per-stage pools let element b+1's embed
        # proceed while element b is in attention/FF.
        psE = ctx.enter_context(tc.tile_pool(name="psE", bufs=psb[0], space="PSUM"))
        psA = ctx.enter_context(tc.tile_pool(name="psA", bufs=psb[1], space="PSUM"))
        psF = ctx.enter_context(tc.tile_pool(name="psF", bufs=psb[2], space="PSUM"))
        psD = ctx.enter_context(tc.tile_pool(name="psD", bufs=psb[3], space="PSUM"))

        # ---- constants / weights into SBUF ----
        ident = const.tile([128, 128], f32, tag="ident")
        make_identity(nc, ident)
        ident_r = const.tile([128, 128], f32r, tag="ident_r")
        nc.vector.tensor_copy(out=ident_r, in_=ident)
        ones = const.tile([1, VN], f32, tag="ones")
        nc.vector.memset(ones, 1.0)
        onescol = const.tile([128, 1], f32, tag="onescol")
        nc.vector.memset(onescol, 1.0)
        onescol_r = const.tile([128, 1], f32r, tag="onescol_r")
        nc.vector.tensor_copy(out=onescol_r, in_=onescol)
        ones_r = const.tile([1, VN], f32r, tag="ones_r")
        nc.vector.tensor_copy(out=ones_r, in_=ones)

        cw = {}
        for nm, h in dp.items():
            t = const.tile(list(h.shape), h.dtype, tag=f"w_{nm}")
            nc.sync.dma_start(out=t, in_=h[:])
            cw[nm] = t

        zeros_r = cw["zeros99"]
        mask_bt = const.tile([bpc, G], f32, tag="mask_bt")
        nc.sync.dma_start(out=mask_bt, in_=obs[:, :, 8])
        lv_bt = const.tile([bpc, LH], f32, tag="lv_bt")
        nc.sync.dma_start(out=lv_bt, in_=obs[:, IH:IH + LH, 8])

        # mask columns: maskT_sb[:, t*bpc + b] = mask of element b, g-chunk t
        mkp = psD.tile([128, 3 * bpc], f32, tag="psD", name="mkp")
        for t in range(3):
            g0, g1 = GC[t]
            nc.tensor.transpose(mkp[:g1 - g0, t * bpc:(t + 1) * bpc],
                                mask_bt[:, g0:g1], ident[:bpc, :bpc])
        maskT_sb = const.tile([128, 3 * bpc], f32, tag="maskT_sb")
        nc.vector.tensor_copy(out=maskT_sb, in_=mkp)

        compat_a = const.tile([56, max(bpc, 2)], f32, tag="compat_a")
        compat_b = const.tile([44, max(bpc, 2)], f32, tag="compat_b")

        def ecopy(eng, out, in_):
            if eng is nc.scalar:
                nc.scalar.copy(out=out, in_=in_)
            else:
                eng.tensor_copy(out=out, in_=in_)

        def mm(out, lhsT, rhs, start, stop):
            nc.tensor.matmul(out, lhsT, rhs, start=start, stop=stop)

        # ========== per-element stages, emitted with pipeline skew ==========
        # Eight stages per element; emission order interleaves elements
        # (deepest stage first each tick) so the list scheduler's priorities
        # let element b+1's early stages fill engine gaps in element b's
        # serial chains.
        def s_embed(b, sv):
            xa = st.tile([20, VN], f32, tag="xa")
            nc.vector.memset(xa, 0.0)
            nc.sync.dma_start(out=xa[0:6, :IH],
                              in_=obs[b, 0:IH, 0:6].rearrange("g f -> f g"))
            nc.sync.dma_start(out=xa[6:14, IH:IH + LH],
                              in_=obs[b, IH:IH + LH, 0:8].rearrange("g f -> f g"))
            nc.sync.dma_start(out=xa[14:20, IH + LH:G],
                              in_=obs[b, IH + LH:G, 0:6].rearrange("g f -> f g"))

            z1 = psE.tile([32, VN], f32, tag="psE", name="z1")
            mm(z1, cw["w1aug"], xa, True, not has_bias)
            if has_bias:
                mm(z1, cw["b1s"], cw["bias_ind"], False, True)

            lr = st.tile([HID, VN], f32r, tag="lr")
            if sv["bb"] < ST_BUFS:
                # zero each rotating buffer once; the written block pattern
                # is identical for every element so zeros persist
                nc.vector.tensor_copy(out=lr, in_=zeros_r)
            small1 = st.tile([32, VN], f32, tag="small1")
            nc.scalar.activation(small1, z1, AF.Identity, scale=0.01)
            nc.vector.tensor_tensor(out=lr[0:32, 0:IH], in0=z1[:, 0:IH],
                                    in1=small1[:, 0:IH], op=ALU.max)
            nc.vector.tensor_tensor(out=lr[32:64, IH:IH + LH],
                                    in0=z1[:, IH:IH + LH],
                                    in1=small1[:, IH:IH + LH], op=ALU.max)
            nc.vector.tensor_tensor(out=lr[64:96, IH + LH:G],
                                    in0=z1[:, IH + LH:G],
                                    in1=small1[:, IH + LH:G], op=ALU.max)
            if has_bias:
                nc.gpsimd.tensor_copy(out=lr[96:HID, :], in_=cw["bias_ind"])
            sv["lr"] = lr

            h0 = stH4.tile([128, 3, E], f32, tag="h0")
            for t in range(3):
                g0, g1 = GC[t]
                gs = g1 - g0
                hp = psE.tile([128, E], f32, tag="psE", name=f"hp{t}")
                mm(hp[:gs, :], lr[:, g0:g1], cw["w2aug"], True, True)
                ecopy(nc.scalar, h0[:gs, t, :], hp[:gs, :])
            sv["h0"] = h0

        def s_qkv(b, sv, l):
            lr, hT = sv.get("lr"), sv.get("hT")
            qT = st.tile([128, 2, VN], f32r, tag=f"qT{l}")
            kT = st.tile([128, 2, VN], f32r, tag=f"kT{l}")
            for m in range(2):
                qp = psE.tile([128, VN], f32, tag="psE", name=f"qp{m}")
                kp = psE.tile([128, VN], f32, tag="psE", name=f"kp{m}")
                if l == 0:
                    mm(qp, cw["wq0"][:, m * 128:(m + 1) * 128], lr, True, True)
                    mm(kp, cw["wk0"][:, m * 128:(m + 1) * 128], lr, True, True)
                else:
                    for k in range(2):
                        mm(qp, cw["wq1"][:, k, m * 128:(m + 1) * 128],
                           hT[:, k, :], k == 0, k == 1)
                        mm(kp, cw["wk1"][:, k, m * 128:(m + 1) * 128],
                           hT[:, k, :], k == 0, k == 1)
                ecopy(nc.scalar, qT[:, m, :], qp)
                ecopy(nc.scalar if ceng[0] else nc.vector, kT[:, m, :], kp)

            v = st.tile([128, 3, E], f32r, tag=f"v{l}")
            for t in range(3):
                g0, g1 = GC[t]
                gs = g1 - g0
                vp = psE.tile([128, E], f32, tag="psE", name=f"vp{t}")
                if l == 0:
                    mm(vp[:gs, :], lr[:, g0:g1], cw["wv0"], True, True)
                else:
                    for k in range(2):
                        mm(vp[:gs, :], hT[:, k, g0:g1], cw["wv1"][:, k, :],
                           k == 0, k == 1)
                ecopy(nc.scalar if (t != 1 or ceng[1]) else nc.vector,
                      v[:gs, t, :], vp[:gs, :])
            sv["qT"], sv["kT"], sv["v"] = qT, kT, v

        def s_attn(b, sv, l):
            qT, kT, v = sv.pop("qT"), sv.pop("kT"), sv.pop("v")
            h_nat = sv["h0"] if l == 0 else sv["h2"]
            aT = stA.tile([128, 3, VN], f32r, tag="aT")
            for tk in range(3):
                k0, k1 = GC[tk]
                ks = k1 - k0
                sp = psA.tile([128, VN], f32, tag="psA", name=f"sp{tk}")
                for m in range(2):
                    mm(sp[:ks, :], kT[:, m, k0:k1], qT[:, m, :],
                       m == 0, m == 1)
                nc.scalar.activation(aT[:ks, tk, :], sp[:ks, :], AF.Exp)

            rs = psA.tile([1, VN], f32, tag="psA", name="rs")
            for tk in range(3):
                k0, k1 = GC[tk]
                ks = k1 - k0
                mm(rs, onescol_r[:ks, :], aT[:ks, tk, :], tk == 0, tk == 2)
            rr = st.tile([1, VN], f32, tag="rr")
            nc.vector.reciprocal(rr, rs)
            rcol = psA.tile([128, 3], f32, tag="psA", name="rcol")
            for t in range(3):
                g0, g1 = GC[t]
                nc.tensor.transpose(rcol[:g1 - g0, t:t + 1], rr[0:1, g0:g1],
                                    ident[:1, :1])
            rinv = st.tile([128, 3], f32, tag="rinv")
            nc.vector.tensor_copy(out=rinv, in_=rcol)

            x1 = stA.tile([128, 3, E], f32, tag="x1")
            for t in range(3):
                g0, g1 = GC[t]
                gs = g1 - g0
                op_ = psA.tile([128, E], f32, tag="psA", name=f"op{t}")
                for tk in range(3):
                    k0, k1 = GC[tk]
                    ks = k1 - k0
                    mm(op_[:gs, :], aT[:ks, tk, g0:g1], v[:ks, tk, :],
                       tk == 0, tk == 2)
                nc.vector.scalar_tensor_tensor(
                    out=x1[:gs, t, :], in0=op_[:gs, :],
                    scalar=rinv[:gs, t:t + 1], in1=h_nat[:gs, t, :],
                    op0=ALU.mult, op1=ALU.add)
            h1 = st.tile([128, 3, E], f32r, tag=f"h1_{l}")
            _layernorm_block(nc, st, x1, h1, f32, i32, AF, ALU)
            sv["h1"] = h1

        def s_ff(b, sv, l):
            h1 = sv.pop("h1")
            h1T = stA.tile([128, 2, VN], f32r, tag="h1T")
            for k in range(2):
                tpk = psF.tile([128, VN], f32r, tag="psF", name=f"tpk{k}")
                for t, (g0, g1) in enumerate(GCT):
                    gs = g1 - g0
                    nc.tensor.transpose(tpk[:, g0:g1],
                                        h1[:gs, t, k * 128:(k + 1) * 128],
                                        ident_r[:gs, :gs])
                ecopy(nc.scalar, h1T[:, k, :], tpk)

            f1 = stA.tile([128, 4, VN], f32r, tag="f1")
            for m in range(4):
                fp = psF.tile([128, VN], f32, tag="psF", name=f"fp{m}")
                for k in range(2):
                    mm(fp, cw[f"wf1{l}"][:, k, m * 128:(m + 1) * 128],
                       h1T[:, k, :], k == 0, k == 1)
                nc.scalar.activation(f1[:, m, :], fp,
                                     AF.Relu, bias=cw[f"bf1{l}"][:, m:m + 1])

            x2 = stA.tile([128, 3, E], f32, tag="x2")
            for t in range(3):
                g0, g1 = GC[t]
                gs = g1 - g0
                f2p = psF.tile([128, E], f32, tag="psF", name=f"f2p{t}")
                for m in range(4):
                    mm(f2p[:gs, :], f1[:, m, g0:g1], cw[f"wf2{l}"][:, m, :],
                       m == 0, m == 3 and not has_bias)
                if has_bias:
                    mm(f2p[:gs, :], ones_r[:, :gs], cw[f"bf2{l}"], False, True)
                nc.vector.scalar_tensor_tensor(
                    out=x2[:gs, t, :], in0=f2p[:gs, :], scalar=1.0,
                    in1=h1[:gs, t, :], op0=ALU.mult, op1=ALU.add)
            h2 = (stH4 if l == 0 else stH3).tile([128, 3, E], f32r, tag=f"h2_{l}")
            _layernorm_block(nc, st, x2, h2, f32, i32, AF, ALU)
            sv["h2"] = h2

            if l < NL - 1:
                hT = st.tile([128, 2, VN], f32r, tag="hT")
                for k in range(2):
                    tpk = psF.tile([128, VN], f32r, tag="psF", name=f"tp2{k}")
                    for t, (g0, g1) in enumerate(GCT):
                        gs = g1 - g0
                        nc.tensor.transpose(tpk[:, g0:g1],
                                            h2[:gs, t, k * 128:(k + 1) * 128],
                                            ident_r[:gs, :gs])
                    ecopy(nc.scalar, hT[:, k, :], tpk)
                sv["hT"] = hT

        def s_dec(b, sv):
            trans = sv.pop("h2")
            for t in range(3):
                g0, g1 = GC[t]
                gs = g1 - g0
                nc.gpsimd.tensor_scalar_mul(
                    trans[:gs, t, :], in0=trans[:gs, t, :],
                    scalar1=maskT_sb[:gs, t * bpc + b:t * bpc + b + 1])

            gep = psD.tile([128, 2], f32, tag="psD", name="gep")
            for k in range(2):
                for t in range(3):
                    g0, g1 = GC[t]
                    gs = g1 - g0
                    mm(gep[:, k:k + 1],
                       trans[:gs, t, k * 128:(k + 1) * 128].bitcast(f32),
                       onescol[:gs, :], t == 0, t == 2)
            ge_sb = st.tile([128, 2], f32, tag="ge_sb")
            nc.vector.tensor_copy(out=ge_sb, in_=gep)

            cp = psD.tile([128, 2], f32, tag="psD", name="cp")
            for m in range(2):
                for k in range(2):
                    mm(cp[:, m:m + 1], cw["mt"][:, k, m * 128:(m + 1) * 128],
                       ge_sb[:, k:k + 1], k == 0, k == 1)
            c_sb = st.tile([128, 2], f32, tag="c_sb")
            nc.scalar.copy(out=c_sb, in_=cp)

            tlp = psD.tile([128, 2, 176], f32r, tag="psD", name="tlp")
            for k in range(2):
                nc.tensor.transpose(tlp[:, k, 0:128],
                                    trans[:128, 1, k * 128:(k + 1) * 128],
                                    ident_r[:128, :128])
                nc.tensor.transpose(tlp[:, k, 128:172],
                                    trans[0:44, 2, k * 128:(k + 1) * 128],
                                    ident_r[:44, :44])
            tTl = stA.tile([128, 2, 176], f32, tag="tTl")
            nc.vector.tensor_copy(out=tTl, in_=tlp)

            cpA = psD.tile([56, 1], f32, tag="psD", name="cpA")
            cpB = psD.tile([44, 1], f32, tag="psD", name="cpB")
            for k in range(2):
                mm(cpA, tTl[:, k, 72:128], c_sb[:, k:k + 1], k == 0, k == 1)
                mm(cpB, tTl[:, k, 128:172], c_sb[:, k:k + 1], k == 0, k == 1)
            nc.scalar.copy(compat_a[:, b:b + 1], cpA)
            nc.scalar.copy(compat_b[:, b:b + 1], cpB)

        stages = [
            s_embed,
            lambda b, sv: s_qkv(b, sv, 0),
            lambda b, sv: s_attn(b, sv, 0),
            lambda b, sv: s_ff(b, sv, 0),
            lambda b, sv: s_qkv(b, sv, 1),
            lambda b, sv: s_attn(b, sv, 1),
            lambda b, sv: s_ff(b, sv, 1),
            s_dec,
        ]
        nelem = nbp if nbp is not None else bpc
        svs = [dict(bb=i) for i in range(nelem)]
        NS = len(stages)
        for tick in range(nelem + NS - 1):
            for s in range(NS - 1, -1, -1):
                b = tick - s
                if 0 <= b < nelem:
                    stages[s](b % bpc, svs[b])

        # ================= batched tail =================
        vl = const.tile([bpc, 1], f32, tag="vl")
        nc.vector.reduce_sum(vl, mask_bt, axis=AX.X)
        ivl2 = const.tile([bpc, 1], f32, tag="ivl2")
        nc.vector.reciprocal(ivl2, vl)
        nc.vector.tensor_scalar_mul(ivl2, in0=ivl2, scalar1=2.0)

        ctp = psD.tile([128, LH], f32, tag="psD", name="ctp")
        nc.tensor.transpose(ctp[:bpc, 0:56], compat_a[:, :bpc], ident[:56, :56])
        nc.tensor.transpose(ctp[:bpc, 56:LH], compat_b[:, :bpc], ident[:44, :44])
        # tanh(z)*CLIP via exp: th = 1 - 2/(e^{2z}+1)
        e2 = const.tile([bpc, LH], f32, tag="e2")
        nc.scalar.activation(e2, ctp[:bpc, :LH], AF.Exp, scale=ivl2)
        d1 = const.tile([bpc, LH], f32, tag="d1")
        nc.vector.tensor_scalar_add(d1, in0=e2, scalar1=1.0)
        rd = const.tile([bpc, LH], f32, tag="rd")
        nc.vector.reciprocal(rd, d1)
        th = const.tile([bpc, LH], f32, tag="th")
        nc.vector.tensor_scalar(out=th, in0=rd, scalar1=-2.0, scalar2=1.0,
                                op0=ALU.mult, op1=ALU.add)
        ex = const.tile([bpc, LH], f32, tag="ex")
        es = const.tile([bpc, 1], f32, tag="es")
        nc.scalar.activation(ex, th, AF.Exp, scale=CLIP, accum_out=es)
        er = const.tile([bpc, 1], f32, tag="er")
        nc.vector.reciprocal(er, es)
        pm = const.tile([bpc, LH], f32, tag="pm")
        nc.vector.tensor_scalar_mul(pm, in0=ex, scalar1=er)
        nc.vector.tensor_tensor(out=pm, in0=pm, in1=lv_bt, op=ALU.mult)
        nc.vector.tensor_scalar_add(pm, in0=pm, scalar1=1e-20)
        rs2 = const.tile([bpc, 1], f32, tag="rs2")
        nc.vector.reduce_sum(rs2, pm, axis=AX.X)
        rr2 = const.tile([bpc, 1], f32, tag="rr2")
        nc.vector.reciprocal(rr2, rs2)
        ob = const.tile([bpc, LH], f32, tag="ob")
        nc.vector.tensor_scalar_mul(ob, in0=pm, scalar1=rr2)
        nc.sync.dma_start(out=out_d[:], in_=ob)

    nc.finalize()
    return nc


def _layernorm_block(nc, st, x, h_out, f32, i32, AF, ALU):
    """LN over free dim (256) of the three g-chunks of x -> h_out.

    Stats on DVE (bn_stats/bn_aggr); rstd = 1/sqrt(var+eps) via DVE-only
    fast-inverse-sqrt (int<->float converting copies for the magic seed +
    2 Newton steps; ~5e-6 rel err) so the ACT engine needs NO sqrt/ln table
    -> whole kernel stays in one act table; normalize on Pool."""
    GC_ = [(0, 128), (128, 256), (256, 301)]
    mv3 = st.tile([128, 3, 2], f32, tag="mv3")
    for t in range(3):
        gs = GC_[t][1] - GC_[t][0]
        st6 = st.tile([128, 6], f32, tag="st6")
        nc.vector.bn_stats(out=st6[:gs], in_=x[:gs, t, :])
        nc.vector.bn_aggr(out=mv3[:gs, t, :], in_=st6[:gs])
    ve = st.tile([128, 3], f32, tag="ve")
    nc.vector.tensor_scalar_add(ve, in0=mv3[:, :, 1], scalar1=1e-5)
    i_f = st.tile([128, 3], f32, tag="i_f")
    nc.vector.tensor_copy(out=i_f, in_=ve.bitcast(i32))
    nc.vector.tensor_scalar(out=i_f, in0=i_f, scalar1=-0.5,
                            scalar2=1597463007.0, op0=ALU.mult, op1=ALU.add)
    y0i = st.tile([128, 3], i32, tag="y0i")
    nc.vector.tensor_copy(out=y0i, in_=i_f)
    y = y0i.bitcast(f32)
    # one Halley step (cubic): rstd = y*(15 - 10w + 3w^2)/8, w = (v+eps)*y^2
    w_ = st.tile([128, 3], f32, tag="w_")
    nc.vector.tensor_tensor(out=w_, in0=y, in1=y, op=ALU.mult)
    nc.vector.tensor_tensor(out=w_, in0=w_, in1=ve, op=ALU.mult)
    p_ = st.tile([128, 3], f32, tag="p_")
    nc.vector.tensor_scalar(out=p_, in0=w_, scalar1=3.0, scalar2=-10.0,
                            op0=ALU.mult, op1=ALU.add)
    nc.vector.tensor_tensor(out=p_, in0=p_, in1=w_, op=ALU.mult)
    nc.vector.tensor_scalar(out=p_, in0=p_, scalar1=15.0, scalar2=0.125,
                            op0=ALU.add, op1=ALU.mult)
    rstd = st.tile([128, 3], f32, tag="rstd")
    nc.vector.tensor_tensor(out=rstd, in0=y, in1=p_, op=ALU.mult)
    for t in range(3):
        gs = GC_[t][1] - GC_[t][0]
        nc.gpsimd.tensor_scalar(
            out=h_out[:gs, t, :], in0=x[:gs, t, :],
            scalar1=mv3[:gs, t, 0:1], scalar2=rstd[:gs, t:t + 1],
            op0=ALU.subtract, op1=ALU.mult,
        )


# ----------------------------------------------------------------------------
# public entry point
# ----------------------------------------------------------------------------
def kernel(**inputs):
    observation = np.asarray(inputs["observation"], np.float32)
    w = _prep_weights(inputs)

    from concourse.bass_utils import run_bass_kernel_spmd

    has_bias = any(
        float(np.abs(np.asarray(inputs[k])).max()) > 0.0
        for k in ("bi1", "bl1", "bn1", "bi2", "bl2", "bn2",
                  "enc_bf1", "enc_bf2"))
    nc = _build(BPC, has_bias=has_bias)
    in_maps = []
    for i in range(NCORES):
        m = {"obs": np.ascontiguousarray(observation[i * BPC:(i + 1) * BPC])}
        m.update(w)
        in_maps.append(m)
    res = run_bass_kernel_spmd(nc, in_maps, list(range(NCORES)))
    out = np.concatenate([res.results[i]["out"] for i in range(NCORES)], axis=0)
    return out.astype(np.float32)


# revision 35
# speedup vs baseline: 1.0007x; 1.0007x over previous
"""Trainium2 Bass kernel for nn_AttentionModel (graph attention encoder + decoder).

Contract: kernel(**inputs) takes FULL unsharded numpy inputs (as produced by
reference.setup_inputs()) and returns the FULL [256, 100] float32 output.
Internally shards the batch (256) across 8 NeuronCores (32 each, pure data
parallel; weights replicated) and runs a fused Bass/Tile kernel per core.

Self-contained: hardcodes all shapes; no sibling imports.

v1 design notes (vs v0 baseline):
- Embedding MLPs done block-diagonally: one [20,304] input tile holds the
  per-type features on disjoint row blocks, so layer-1 is ONE matmul over all
  301 nodes; leaky-relu is done with native ACT Lrelu ops writing straight
  into the block-diagonal [99,304] hidden tile (rows 96:99 hold constant
  type-indicator rows so every per-type bias rides along as extra
  contraction rows).
- Layer-0 q/k/v are folded into the embedding layer-2 weights on the host
  (W2aug @ wq etc.), so h0T never exists; wv@wo is folded into a single
  v' = h @ (wv wo) everywhere (exact by associativity), which removes the
  output projection and lets attention output be computed naturally.
- Scores are computed TRANSPOSED (sT[k,q]) so exp(sT) IS the aT operand the
  AV matmul needs: no attention-matrix transposes. Row sums come from a
  ones-column matmul; 1/rowsum is applied per-q-row fused into the residual
  add (scalar_tensor_tensor).
- LN: bn_stats/bn_aggr stats (DVE), rstd = Exp(-0.5*Ln(var+eps)) (tiny ACT,
  keeps ALL activations in the single natural_log_exp table -> no act-table
  reloads), normalize on the otherwise-idle Pool engine.
- Final tanh is computed from exp (tanh z = 1 - 2/(e^{2z}+1)) to stay in the
  one act table.
- Copies psum->sbuf are balanced across ACT and DVE.
"""

import sys

for _p in ("/opt/trn_rl_repo", "/opt/pypackages"):
    if _p not in sys.path:
        sys.path.append(_p)

import numpy as np
from contextlib import ExitStack

# --- static architecture constants ---
B, IH, IL, LH, E, FFH, NL = 256, 200, 6, 100, 256, 512, 2
G = IH + LH + 1  # 301
CLIP = 10.0
SCALE = 1.0 / 16.0  # 1/sqrt(E)
NCORES = 8
BPC = B // NCORES  # 32 batch elements per core

GC = [(0, 128), (128, 256), (256, 301)]  # g chunks over 301 nodes
GCT = [(0, 128), (128, 256), (256, 302)]  # even-row chunks for f32r transposes
VN = 304  # padded moving width over the node axis
HID = 99  # 3*32 block-diag hidden rows + 3 bias-indicator rows


# ----------------------------------------------------------------------------
# host-side weight packing
# ----------------------------------------------------------------------------
def _tf32(x):
    """Round fp32 array to tfloat32 (10 mantissa bits), round-to-nearest-even."""
    u = np.ascontiguousarray(x, np.float32).view(np.uint32)
    u = (u + 0x0FFF + ((u >> 13) & 1)) & np.uint32(0xFFFFE000)
    return u.view(np.float32)


def _pack_rows(m, nchunk):
    """[nchunk*128, N] -> [128, nchunk, N] with [:, k, :] = m[128k:128(k+1), :]"""
    return np.ascontiguousarray(
        np.stack([m[i * 128:(i + 1) * 128] for i in range(nchunk)], axis=1)
    ).astype(np.float32)


def _prep_weights(inp):
    f32 = np.float32
    w = {}
    # block-diag layer 1: [20, 32] = [wi1; wl1; wn1], bias rows [3, 32]
    w["w1aug"] = np.concatenate(
        [inp["wi1"], inp["wl1"], inp["wn1"]], axis=0).astype(f32)  # [20, 32]
    w["b1s"] = np.stack(
        [inp["bi1"], inp["bl1"], inp["bn1"]], axis=0).astype(f32)  # [3, 32]
    # type indicator rows [3, 304] (1 on that type's node columns)
    bi = np.zeros((3, VN), f32)
    bi[0, :IH] = 1.0
    bi[1, IH:IH + LH] = 1.0
    bi[2, IH + LH:G] = 1.0
    w["bias_ind"] = bi
    # block-diag layer 2 (+ bias rows 96:99): [99, 256]
    w2aug = np.zeros((HID, E), f32)
    w2aug[0:32] = inp["wi2"]
    w2aug[32:64] = inp["wl2"]
    w2aug[64:96] = inp["wn2"]
    w2aug[96] = inp["bi2"]
    w2aug[97] = inp["bl2"]
    w2aug[98] = inp["bn2"]
    w["w2aug"] = _tf32(w2aug)
    # layer-0 q/k/v folded through the embedding layer 2
    wq0 = inp["enc_wq"][0] * SCALE
    wk0 = inp["enc_wk"][0]
    wv0 = inp["enc_wv"][0] @ inp["enc_wo"][0]
    w["wq0"] = _tf32(w2aug @ wq0)                               # [99, 256]
    w["wk0"] = _tf32(w2aug @ wk0)
    w["wv0"] = _tf32(w2aug @ wv0)
    # layer-1 projections (lhsT/rhs chunk packs)
    w["wq1"] = _tf32(_pack_rows(inp["enc_wq"][1] * SCALE, 2))   # [128,2,256]
    w["wk1"] = _tf32(_pack_rows(inp["enc_wk"][1], 2))
    w["wv1"] = _tf32(_pack_rows(inp["enc_wv"][1] @ inp["enc_wo"][1], 2))
    for l in range(NL):
        w[f"wf1{l}"] = _tf32(_pack_rows(inp["enc_wf1"][l], 2))  # [128,2,512]
        w[f"bf1{l}"] = np.ascontiguousarray(
            inp["enc_bf1"][l].reshape(4, 128).T).astype(f32)    # [128, 4]
        w[f"wf2{l}"] = _tf32(_pack_rows(inp["enc_wf2"][l], 4))  # [128,4,256]
        w[f"bf2{l}"] = _tf32(inp["enc_bf2"][l].reshape(1, E))
    # decoder fused matrix: compat = trans_leaf . (M @ ge), M = Wpn_E @ Wfc.T
    MT = (inp["w_fc"] @ inp["w_pn"][:, :E].T) * SCALE
    w["mt"] = _tf32(_pack_rows(MT, 2))                          # [128,2,256]
    w["zeros99"] = np.zeros((HID, VN), f32)
    return w


# ----------------------------------------------------------------------------
# numpy mirror of the device computation (for algebra validation)
# ----------------------------------------------------------------------------
def _numpy_mirror(observation, w):
    obs = observation.astype(np.float32)
    nb = obs.shape[0]
    out = np.zeros((nb, LH), np.float32)

    def lrelu(x):
        return np.maximum(x, 0.01 * x)

    def ln(x):
        m = x.mean(-1, keepdims=True)
        v = x.var(-1, keepdims=True)
        return (x - m) / np.sqrt(v + 1e-5)

    bi = w["bias_ind"][:, :G]  # [3, 301]
    for b in range(nb):
        # block-diagonal feature tile [20, 301]
        xa = np.zeros((20, G), np.float32)
        xa[0:6, :IH] = obs[b, :IH, :6].T
        xa[6:14, IH:IH + LH] = obs[b, IH:IH + LH, :8].T
        xa[14:20, IH + LH:] = obs[b, IH + LH:, :6].T
        z1 = w["w1aug"].T @ xa + w["b1s"].T @ bi               # [32, 301]
        lr = np.zeros((HID, G), np.float32)
        lr[0:32, :IH] = lrelu(z1[:, :IH])
        lr[32:64, IH:IH + LH] = lrelu(z1[:, IH:IH + LH])
        lr[64:96, IH + LH:] = lrelu(z1[:, IH + LH:])
        lr[96:99] = bi
        h = (lr.T @ w["w2aug"])                                 # [301, 256]

        for l in range(NL):
            if l == 0:
                qT = w["wq0"].T @ lr                            # [256, 301]
                kT = w["wk0"].T @ lr
                v = (lr.T @ w["wv0"])                           # [301, 256]
            else:
                wq = np.concatenate([w["wq1"][:, 0], w["wq1"][:, 1]], 0)
                wk = np.concatenate([w["wk1"][:, 0], w["wk1"][:, 1]], 0)
                wv = np.concatenate([w["wv1"][:, 0], w["wv1"][:, 1]], 0)
                qT = (h @ wq).T
                kT = (h @ wk).T
                v = h @ wv
            sT = kT.T @ qT                                      # [301k, 301q]
            aT = np.exp(sT)
            rinv = 1.0 / aT.sum(0)                              # per q
            o = (aT.T @ v) * rinv[:, None]                      # [301q, 256]
            x1 = h + o
            h = ln(x1)
            wf1 = np.concatenate([w[f"wf1{l}"][:, 0], w[f"wf1{l}"][:, 1]], 0)
            wf2 = np.concatenate([w[f"wf2{l}"][:, k] for k in range(4)], 0)
            bf1 = w[f"bf1{l}"].T.reshape(-1)
            f = np.maximum(h @ wf1 + bf1, 0.0) @ wf2 + w[f"bf2{l}"]
            h = ln(h + f)

        mask = obs[b, :, 8]
        trans = h * mask[:, None]
        ge = trans.sum(0)
        MT = np.concatenate([w["mt"][:, 0], w["mt"][:, 1]], 0)
        c = MT.T @ ge
        compat = trans[IH:IH + LH] @ c
        vlen = mask.sum()
        z = compat / vlen
        th = 1.0 - 2.0 / (np.exp(2.0 * z) + 1.0)                # tanh via exp
        ee = np.exp(th * CLIP)
        p = ee / ee.sum()
        lv = obs[b, IH:IH + LH, 8]
        masked = p * lv + 1e-20
        out[b] = masked / masked.sum()
    return out


# ----------------------------------------------------------------------------
# the Bass/Tile kernel
# ----------------------------------------------------------------------------
def _build(bpc, dbg=False, nbp=None, psb=(3, 2, 2, 1), has_bias=True, ceng=(1, 0), bufs=(3, 2, 5, 4)):
    import concourse.bass as bass
    import concourse.mybir as mybir
    import concourse.tile as tile
    from concourse import bacc
    from concourse.masks import make_identity

    f32 = mybir.dt.float32
    f32r = mybir.dt.float32r
    i32 = mybir.dt.int32
    AF = mybir.ActivationFunctionType
    ALU = mybir.AluOpType
    AX = mybir.AxisListType

    def r(ap):
        return ap.bitcast(f32r)

    nc = bacc.Bacc(None, target_bir_lowering=False)

    obs = nc.declare_dram_parameter("obs", [bpc, G, 9], f32, isOutput=False)
    dp = {}
    F32_WEIGHTS = {"w1aug", "b1s", "bias_ind", "bf10", "bf11", "mt"}
    for nm, shp in (
        ("w1aug", [20, 32]), ("b1s", [3, 32]), ("bias_ind", [3, VN]),
        ("w2aug", [HID, E]), ("wq0", [HID, E]), ("wk0", [HID, E]),
        ("wv0", [HID, E]),
        ("wq1", [128, 2, E]), ("wk1", [128, 2, E]), ("wv1", [128, 2, E]),
        ("wf10", [128, 2, FFH]), ("bf10", [128, 4]), ("wf20", [128, 4, E]),
        ("bf20", [1, E]),
        ("wf11", [128, 2, FFH]), ("bf11", [128, 4]), ("wf21", [128, 4, E]),
        ("bf21", [1, E]),
        ("mt", [128, 2, E]),  # fp32 (moving-1 matmuls)
        ("zeros99", [HID, VN]),
    ):
        wdt = f32 if nm in F32_WEIGHTS else f32r
        dp[nm] = nc.declare_dram_parameter(nm, shp, wdt, isOutput=False)
    out_d = nc.declare_dram_parameter("out", [bpc, LH], f32, isOutput=True)

    with tile.TileContext(nc) as tc, ExitStack() as ctx:
        const = ctx.enter_context(tc.tile_pool(name="const", bufs=1))
        ST_BUFS = bufs[0]
        st = ctx.enter_context(tc.tile_pool(name="st", bufs=ST_BUFS))
        stA = ctx.enter_context(tc.tile_pool(name="stA", bufs=bufs[1]))
        stH4 = ctx.enter_context(tc.tile_pool(name="stH4", bufs=bufs[2]))
        stH3 = ctx.enter_context(tc.tile_pool(name="stH3", bufs=bufs[3]))
        # Per-stage PSUM pools (8 banks total). A single shared-tag pool
        # makes every element's ~30 psum tiles queue FIFO behind each other,
        # serializing elements; per-stage pools let element b+1's embed
        # proceed while element b is in attention/FF.
        psE = ctx.enter_context(tc.tile_pool(name="psE", bufs=psb[0], space="PSUM"))
        psA = ctx.enter_context(tc.tile_pool(name="psA", bufs=psb[1], space="PSUM"))
        psF = ctx.enter_context(tc.tile_pool(name="psF", bufs=psb[2], space="PSUM"))
        psD = ctx.enter_context(tc.tile_pool(name="psD", bufs=psb[3], space="PSUM"))

        # ---- constants / weights into SBUF ----
        ident = const.tile([128, 128], f32, tag="ident")
        make_identity(nc, ident)
        ident_r = const.tile([128, 128], f32r, tag="ident_r")
        nc.vector.tensor_copy(out=ident_r, in_=ident)
        ones = const.tile([1, VN], f32, tag="ones")
        nc.vector.memset(ones, 1.0)
        onescol = const.tile([128, 1], f32, tag="onescol")
        nc.vector.memset(onescol, 1.0)
        onescol_r = const.tile([128, 1], f32r, tag="onescol_r")
        nc.vector.tensor_copy(out=onescol_r, in_=onescol)
        ones_r = const.tile([1, VN], f32r, tag="ones_r")
        nc.vector.tensor_copy(out=ones_r, in_=ones)

        cw = {}
        for nm, h in dp.items():
            t = const.tile(list(h.shape), h.dtype, tag=f"w_{nm}")
            nc.sync.dma_start(out=t, in_=h[:])
            cw[nm] = t

        zeros_r = cw["zeros99"]
        mask_bt = const.tile([bpc, G], f32, tag="mask_bt")
        nc.sync.dma_start(out=mask_bt, in_=obs[:, :, 8])
        lv_bt = const.tile([bpc, LH], f32, tag="lv_bt")
        nc.sync.dma_start(out=lv_bt, in_=obs[:, IH:IH + LH, 8])

        # mask columns: maskT_sb[:, t*bpc + b] = mask of element b, g-chunk t
        mkp = psD.tile([128, 3 * bpc], f32, tag="psD", name="mkp")
        for t in range(3):
            g0, g1 = GC[t]
            nc.tensor.transpose(mkp[:g1 - g0, t * bpc:(t + 1) * bpc],
                                mask_bt[:, g0:g1], ident[:bpc, :bpc])
        maskT_sb = const.tile([128, 3 * bpc], f32, tag="maskT_sb")
        nc.vector.tensor_copy(out=maskT_sb, in_=mkp)

        compat_a = const.tile([56, max(bpc, 2)], f32, tag="compat_a")
        compat_b = const.tile([44, max(bpc, 2)], f32, tag="compat_b")

        def ecopy(eng, out, in_):
            if eng is nc.scalar:
                nc.scalar.copy(out=out, in_=in_)
            else:
                eng.tensor_copy(out=out, in_=in_)

        def mm(out, lhsT, rhs, start, stop):
            nc.tensor.matmul(out, lhsT, rhs, start=start, stop=stop)

        # ========== per-element stages, emitted with pipeline skew ==========
        # Eight stages per element; emission order interleaves elements
        # (deepest stage first each tick) so the list scheduler's priorities
        # let element b+1's early stages fill engine gaps in element b's
        # serial chains.
        def s_embed(b, sv):
            xa = st.tile([20, VN], f32, tag="xa")
            if sv["bb"] < ST_BUFS:
                # zero each rotating buffer once; DMA writes the same block
                # pattern every element so the complement stays zero
                nc.vector.memset(xa, 0.0)
            nc.sync.dma_start(out=xa[0:6, :IH],
                              in_=obs[b, 0:IH, 0:6].rearrange("g f -> f g"))
            nc.sync.dma_start(out=xa[6:14, IH:IH + LH],
                              in_=obs[b, IH:IH + LH, 0:8].rearrange("g f -> f g"))
            nc.sync.dma_start(out=xa[14:20, IH + LH:G],
                              in_=obs[b, IH + LH:G, 0:6].rearrange("g f -> f g"))

            z1 = psE.tile([32, VN], f32, tag="psE", name="z1")
            mm(z1, cw["w1aug"], xa, True, not has_bias)
            if has_bias:
                mm(z1, cw["b1s"], cw["bias_ind"], False, True)

            lr = st.tile([HID, VN], f32r, tag="lr")
            if sv["bb"] < ST_BUFS:
                # zero each rotating buffer once; the written block pattern
                # is identical for every element so zeros persist
                nc.vector.tensor_copy(out=lr, in_=zeros_r)
            small1 = st.tile([32, VN], f32, tag="small1")
            nc.scalar.activation(small1, z1, AF.Identity, scale=0.01)
            nc.vector.tensor_tensor(out=lr[0:32, 0:IH], in0=z1[:, 0:IH],
                                    in1=small1[:, 0:IH], op=ALU.max)
            nc.vector.tensor_tensor(out=lr[32:64, IH:IH + LH],
                                    in0=z1[:, IH:IH + LH],
                                    in1=small1[:, IH:IH + LH], op=ALU.max)
            nc.vector.tensor_tensor(out=lr[64:96, IH + LH:G],
                                    in0=z1[:, IH + LH:G],
                                    in1=small1[:, IH + LH:G], op=ALU.max)
            if has_bias:
                nc.gpsimd.tensor_copy(out=lr[96:HID, :], in_=cw["bias_ind"])
            sv["lr"] = lr

            h0 = stH4.tile([128, 3, E], f32, tag="h0")
            for t in range(3):
                g0, g1 = GC[t]
                gs = g1 - g0
                hp = psE.tile([128, E], f32, tag="psE", name=f"hp{t}")
                mm(hp[:gs, :], lr[:, g0:g1], cw["w2aug"], True, True)
                ecopy(nc.scalar, h0[:gs, t, :], hp[:gs, :])
            sv["h0"] = h0

        def s_qkv(b, sv, l):
            lr, hT = sv.get("lr"), sv.get("hT")
            qT = st.tile([128, 2, VN], f32r, tag=f"qT{l}")
            kT = st.tile([128, 2, VN], f32r, tag=f"kT{l}")
            for m in range(2):
                qp = psE.tile([128, VN], f32, tag="psE", name=f"qp{m}")
                kp = psE.tile([128, VN], f32, tag="psE", name=f"kp{m}")
                if l == 0:
                    mm(qp, cw["wq0"][:, m * 128:(m + 1) * 128], lr, True, True)
                    mm(kp, cw["wk0"][:, m * 128:(m + 1) * 128], lr, True, True)
                else:
                    for k in range(2):
                        mm(qp, cw["wq1"][:, k, m * 128:(m + 1) * 128],
                           hT[:, k, :], k == 0, k == 1)
                        mm(kp, cw["wk1"][:, k, m * 128:(m + 1) * 128],
                           hT[:, k, :], k == 0, k == 1)
                ecopy(nc.scalar, qT[:, m, :], qp)
                ecopy(nc.scalar if ceng[0] else nc.vector, kT[:, m, :], kp)

            v = st.tile([128, 3, E], f32r, tag=f"v{l}")
            for t in range(3):
                g0, g1 = GC[t]
                gs = g1 - g0
                vp = psE.tile([128, E], f32, tag="psE", name=f"vp{t}")
                if l == 0:
                    mm(vp[:gs, :], lr[:, g0:g1], cw["wv0"], True, True)
                else:
                    for k in range(2):
                        mm(vp[:gs, :], hT[:, k, g0:g1], cw["wv1"][:, k, :],
                           k == 0, k == 1)
                ecopy(nc.scalar if (t != 1 or ceng[1]) else nc.vector,
                      v[:gs, t, :], vp[:gs, :])
            sv["qT"], sv["kT"], sv["v"] = qT, kT, v

        def s_attn(b, sv, l):
            qT, kT, v = sv.pop("qT"), sv.pop("kT"), sv.pop("v")
            h_nat = sv["h0"] if l == 0 else sv["h2"]
            aT = stA.tile([128, 3, VN], f32r, tag="aT")
            for tk in range(3):
                k0, k1 = GC[tk]
                ks = k1 - k0
                sp = psA.tile([128, VN], f32, tag="psA", name=f"sp{tk}")
                for m in range(2):
                    mm(sp[:ks, :], kT[:, m, k0:k1], qT[:, m, :],
                       m == 0, m == 1)
                nc.scalar.activation(aT[:ks, tk, :], sp[:ks, :], AF.Exp)

            rs = psA.tile([1, VN], f32, tag="psA", name="rs")
            for tk in range(3):
                k0, k1 = GC[tk]
                ks = k1 - k0
                mm(rs, onescol_r[:ks, :], aT[:ks, tk, :], tk == 0, tk == 2)
            rr = st.tile([1, VN], f32, tag="rr")
            nc.vector.reciprocal(rr, rs)
            rcol = psA.tile([128, 3], f32, tag="psA", name="rcol")
            for t in range(3):
                g0, g1 = GC[t]
                nc.tensor.transpose(rcol[:g1 - g0, t:t + 1], rr[0:1, g0:g1],
                                    ident[:1, :1])
            rinv = st.tile([128, 3], f32, tag="rinv")
            nc.vector.tensor_copy(out=rinv, in_=rcol)

            x1 = stA.tile([128, 3, E], f32, tag="x1")
            for t in range(3):
                g0, g1 = GC[t]
                gs = g1 - g0
                op_ = psA.tile([128, E], f32, tag="psA", name=f"op{t}")
                for tk in range(3):
                    k0, k1 = GC[tk]
                    ks = k1 - k0
                    mm(op_[:gs, :], aT[:ks, tk, g0:g1], v[:ks, tk, :],
                       tk == 0, tk == 2)
                nc.vector.scalar_tensor_tensor(
                    out=x1[:gs, t, :], in0=op_[:gs, :],
                    scalar=rinv[:gs, t:t + 1], in1=h_nat[:gs, t, :],
                    op0=ALU.mult, op1=ALU.add)
            h1 = st.tile([128, 3, E], f32r, tag=f"h1_{l}")
            _layernorm_block(nc, st, x1, h1, f32, i32, AF, ALU)
            sv["h1"] = h1

        def s_ff(b, sv, l):
            h1 = sv.pop("h1")
            h1T = stA.tile([128, 2, VN], f32r, tag="h1T")
            for k in range(2):
                tpk = psF.tile([128, VN], f32r, tag="psF", name=f"tpk{k}")
                for t, (g0, g1) in enumerate(GCT):
                    gs = g1 - g0
                    nc.tensor.transpose(tpk[:, g0:g1],
                                        h1[:gs, t, k * 128:(k + 1) * 128],
                                        ident_r[:gs, :gs])
                ecopy(nc.scalar, h1T[:, k, :], tpk)

            f1 = stA.tile([128, 4, VN], f32r, tag="f1")
            for m in range(4):
                fp = psF.tile([128, VN], f32, tag="psF", name=f"fp{m}")
                for k in range(2):
                    mm(fp, cw[f"wf1{l}"][:, k, m * 128:(m + 1) * 128],
                       h1T[:, k, :], k == 0, k == 1)
                nc.scalar.activation(f1[:, m, :], fp,
                                     AF.Relu, bias=cw[f"bf1{l}"][:, m:m + 1])

            x2 = stA.tile([128, 3, E], f32, tag="x2")
            for t in range(3):
                g0, g1 = GC[t]
                gs = g1 - g0
                f2p = psF.tile([128, E], f32, tag="psF", name=f"f2p{t}")
                for m in range(4):
                    mm(f2p[:gs, :], f1[:, m, g0:g1], cw[f"wf2{l}"][:, m, :],
                       m == 0, m == 3 and not has_bias)
                if has_bias:
                    mm(f2p[:gs, :], ones_r[:, :gs], cw[f"bf2{l}"], False, True)
                nc.vector.scalar_tensor_tensor(
                    out=x2[:gs, t, :], in0=f2p[:gs, :], scalar=1.0,
                    in1=h1[:gs, t, :], op0=ALU.mult, op1=ALU.add)
            h2 = (stH4 if l == 0 else stH3).tile([128, 3, E], f32r, tag=f"h2_{l}")
            _layernorm_block(nc, st, x2, h2, f32, i32, AF, ALU)
            sv["h2"] = h2

            if l < NL - 1:
                hT = st.tile([128, 2, VN], f32r, tag="hT")
                for k in range(2):
                    tpk = psF.tile([128, VN], f32r, tag="psF", name=f"tp2{k}")
                    for t, (g0, g1) in enumerate(GCT):
                        gs = g1 - g0
                        nc.tensor.transpose(tpk[:, g0:g1],
                                            h2[:gs, t, k * 128:(k + 1) * 128],
                                            ident_r[:gs, :gs])
                    ecopy(nc.scalar, hT[:, k, :], tpk)
                sv["hT"] = hT

        def s_dec(b, sv):
            trans = sv.pop("h2")
            for t in range(3):
                g0, g1 = GC[t]
                gs = g1 - g0
                nc.gpsimd.tensor_scalar_mul(
                    trans[:gs, t, :], in0=trans[:gs, t, :],
                    scalar1=maskT_sb[:gs, t * bpc + b:t * bpc + b + 1])

            gep = psD.tile([128, 2], f32, tag="psD", name="gep")
            for k in range(2):
                for t in range(3):
                    g0, g1 = GC[t]
                    gs = g1 - g0
                    mm(gep[:, k:k + 1],
                       trans[:gs, t, k * 128:(k + 1) * 128].bitcast(f32),
                       onescol[:gs, :], t == 0, t == 2)
            ge_sb = st.tile([128, 2], f32, tag="ge_sb")
            nc.vector.tensor_copy(out=ge_sb, in_=gep)

            cp = psD.tile([128, 2], f32, tag="psD", name="cp")
            for m in range(2):
                for k in range(2):
                    mm(cp[:, m:m + 1], cw["mt"][:, k, m * 128:(m + 1) * 128],
                       ge_sb[:, k:k + 1], k == 0, k == 1)
            c_sb = st.tile([128, 2], f32, tag="c_sb")
            nc.scalar.copy(out=c_sb, in_=cp)

            tlp = psD.tile([128, 2, 176], f32r, tag="psD", name="tlp")
            for k in range(2):
                nc.tensor.transpose(tlp[:, k, 0:128],
                                    trans[:128, 1, k * 128:(k + 1) * 128],
                                    ident_r[:128, :128])
                nc.tensor.transpose(tlp[:, k, 128:172],
                                    trans[0:44, 2, k * 128:(k + 1) * 128],
                                    ident_r[:44, :44])
            tTl = stA.tile([128, 2, 176], f32, tag="tTl")
            nc.vector.tensor_copy(out=tTl, in_=tlp)

            cpA = psD.tile([56, 1], f32, tag="psD", name="cpA")
            cpB = psD.tile([44, 1], f32, tag="psD", name="cpB")
            for k in range(2):
                mm(cpA, tTl[:, k, 72:128], c_sb[:, k:k + 1], k == 0, k == 1)
                mm(cpB, tTl[:, k, 128:172], c_sb[:, k:k + 1], k == 0, k == 1)
            nc.scalar.copy(compat_a[:, b:b + 1], cpA)
            nc.scalar.copy(compat_b[:, b:b + 1], cpB)

        stages = [
            s_embed,
            lambda b, sv: s_qkv(b, sv, 0),
            lambda b, sv: s_attn(b, sv, 0),
            lambda b, sv: s_ff(b, sv, 0),
            lambda b, sv: s_qkv(b, sv, 1),
            lambda b, sv: s_attn(b, sv, 1),
            lambda b, sv: s_ff(b, sv, 1),
            s_dec,
        ]
        nelem = nbp if nbp is not None else bpc
        svs = [dict(bb=i) for i in range(nelem)]
        NS = len(stages)
        for tick in range(nelem + NS - 1):
            for s in range(NS - 1, -1, -1):
                b = tick - s
                if 0 <= b < nelem:
                    stages[s](b % bpc, svs[b])

        # ================= batched tail =================
        vl = const.tile([bpc, 1], f32, tag="vl")
        nc.vector.reduce_sum(vl, mask_bt, axis=AX.X)
        ivl2 = const.tile([bpc, 1], f32, tag="ivl2")
        nc.vector.reciprocal(ivl2, vl)
        nc.vector.tensor_scalar_mul(ivl2, in0=ivl2, scalar1=2.0)

        ctp = psD.tile([128, LH], f32, tag="psD", name="ctp")
        nc.tensor.transpose(ctp[:bpc, 0:56], compat_a[:, :bpc], ident[:56, :56])
        nc.tensor.transpose(ctp[:bpc, 56:LH], compat_b[:, :bpc], ident[:44, :44])
        # tanh(z)*CLIP via exp: th = 1 - 2/(e^{2z}+1)
        e2 = const.tile([bpc, LH], f32, tag="e2")
        nc.scalar.activation(e2, ctp[:bpc, :LH], AF.Exp, scale=ivl2)
        d1 = const.tile([bpc, LH], f32, tag="d1")
        nc.vector.tensor_scalar_add(d1, in0=e2, scalar1=1.0)
        rd = const.tile([bpc, LH], f32, tag="rd")
        nc.vector.reciprocal(rd, d1)
        th = const.tile([bpc, LH], f32, tag="th")
        nc.vector.tensor_scalar(out=th, in0=rd, scalar1=-2.0, scalar2=1.0,
                                op0=ALU.mult, op1=ALU.add)
        ex = const.tile([bpc, LH], f32, tag="ex")
        es = const.tile([bpc, 1], f32, tag="es")
        nc.scalar.activation(ex, th, AF.Exp, scale=CLIP, accum_out=es)
        er = const.tile([bpc, 1], f32, tag="er")
        nc.vector.reciprocal(er, es)
        pm = const.tile([bpc, LH], f32, tag="pm")
        nc.vector.tensor_scalar_mul(pm, in0=ex, scalar1=er)
        nc.vector.tensor_tensor(out=pm, in0=pm, in1=lv_bt, op=ALU.mult)
        nc.vector.tensor_scalar_add(pm, in0=pm, scalar1=1e-20)
        rs2 = const.tile([bpc, 1], f32, tag="rs2")
        nc.vector.reduce_sum(rs2, pm, axis=AX.X)
        rr2 = const.tile([bpc, 1], f32, tag="rr2")
        nc.vector.reciprocal(rr2, rs2)
        ob = const.tile([bpc, LH], f32, tag="ob")
        nc.vector.tensor_scalar_mul(ob, in0=pm, scalar1=rr2)
        nc.sync.dma_start(out=out_d[:], in_=ob)

    nc.finalize()
    return nc


def _layernorm_block(nc, st, x, h_out, f32, i32, AF, ALU):
    """LN over free dim (256) of the three g-chunks of x -> h_out.

    Stats on DVE (bn_stats/bn_aggr); rstd = 1/sqrt(var+eps) via DVE-only
    fast-inverse-sqrt (int<->float converting copies for the magic seed +
    2 Newton steps; ~5e-6 rel err) so the ACT engine needs NO sqrt/ln table
    -> whole kernel stays in one act table; normalize on Pool."""
    GC_ = [(0, 128), (128, 256), (256, 301)]
    mv3 = st.tile([128, 3, 2], f32, tag="mv3")
    for t in range(3):
        gs = GC_[t][1] - GC_[t][0]
        st6 = st.tile([128, 6], f32, tag="st6")
        nc.vector.bn_stats(out=st6[:gs], in_=x[:gs, t, :])
        nc.vector.bn_aggr(out=mv3[:gs, t, :], in_=st6[:gs])
    ve = st.tile([128, 3], f32, tag="ve")
    nc.vector.tensor_scalar_add(ve, in0=mv3[:, :, 1], scalar1=1e-5)
    i_f = st.tile([128, 3], f32, tag="i_f")
    nc.vector.tensor_copy(out=i_f, in_=ve.bitcast(i32))
    nc.vector.tensor_scalar(out=i_f, in0=i_f, scalar1=-0.5,
                            scalar2=1597463007.0, op0=ALU.mult, op1=ALU.add)
    y0i = st.tile([128, 3], i32, tag="y0i")
    nc.vector.tensor_copy(out=y0i, in_=i_f)
    y = y0i.bitcast(f32)
    # one Halley step (cubic): rstd = y*(15 - 10w + 3w^2)/8, w = (v+eps)*y^2
    w_ = st.tile([128, 3], f32, tag="w_")
    nc.vector.tensor_tensor(out=w_, in0=y, in1=y, op=ALU.mult)
    nc.vector.tensor_tensor(out=w_, in0=w_, in1=ve, op=ALU.mult)
    p_ = st.tile([128, 3], f32, tag="p_")
    nc.vector.tensor_scalar(out=p_, in0=w_, scalar1=3.0, scalar2=-10.0,
                            op0=ALU.mult, op1=ALU.add)
    nc.vector.tensor_tensor(out=p_, in0=p_, in1=w_, op=ALU.mult)
    nc.vector.tensor_scalar(out=p_, in0=p_, scalar1=15.0, scalar2=0.125,
                            op0=ALU.add, op1=ALU.mult)
    rstd = st.tile([128, 3], f32, tag="rstd")
    nc.vector.tensor_tensor(out=rstd, in0=y, in1=p_, op=ALU.mult)
    for t in range(3):
        gs = GC_[t][1] - GC_[t][0]
        nc.gpsimd.tensor_scalar(
            out=h_out[:gs, t, :], in0=x[:gs, t, :],
            scalar1=mv3[:gs, t, 0:1], scalar2=rstd[:gs, t:t + 1],
            op0=ALU.subtract, op1=ALU.mult,
        )


# ----------------------------------------------------------------------------
# public entry point
# ----------------------------------------------------------------------------
def kernel(**inputs):
    observation = np.asarray(inputs["observation"], np.float32)
    w = _prep_weights(inputs)

    from concourse.bass_utils import run_bass_kernel_spmd

    has_bias = any(
        float(np.abs(np.asarray(inputs[k])).max()) > 0.0
        for k in ("bi1", "bl1", "bn1", "bi2", "bl2", "bn2",
                  "enc_bf1", "enc_bf2"))
    nc = _build(BPC, has_bias=has_bias)
    in_maps = []
    for i in range(NCORES):
        m = {"obs": np.ascontiguousarray(observation[i * BPC:(i + 1) * BPC])}
        m.update(w)
        in_maps.append(m)
    res = run_bass_kernel_spmd(nc, in_maps, list(range(NCORES)))
    out = np.concatenate([res.results[i]["out"] for i in range(NCORES)], axis=0)
    return out.astype(np.float32)
